# revision 29
# baseline (speedup 1.0000x reference)
"""Causal multi-head self-attention (B=4, T=2048, C=1024, H=16) on 8 TRN2 NeuronCores.

Sharding: core = b*2 + g  (b = batch 0..3, g = head-group 0..1 of 8 heads each).
Data parallel over batch; tensor parallel over heads (column-parallel W_attn,
row-parallel W_proj). Each core returns a partial (T, C) output; the host sums
the two partials per batch (the TP all-reduce happens in the unshard step).

Per-core device kernel (bf16 matmuls, f32 accumulation), per 512-wide q chunk:
  1. qT/kT projection with heads on partitions; head pairs share a 128-row tile
  2. v_aug projection in natural [t, c] layout with an all-ones column per head
     (the ones column turns the softmax denominator into row 64 of the y^T psum)
  3. attention in transposed [s, q] layout -- no transposes anywhere:
       S^T block = (kT block)^T @ qT chunk  (head-even rows 0:64 / head-odd rows
       64:128 of the PE array run concurrently: disjoint row groups)
       E = exp(S/8) on ScalarE (no max-subtraction: |scores|/8 < ~7)
       causal mask = precomputed multiplicative 0/1 tile on diagonal blocks
       y^T_aug accumulates v_aug^T @ E over s blocks in PSUM
  4. row-parallel output projection of the finished q chunk.

v2 scheduling: the attention loop runs in per-kv-block units holding one
[128, 1024] S psum tile (head A in cols 0:512, head B in 512:1024, one exp
ACT instruction covers both).  Units are software-pipelined one ahead (the
S matmuls of unit i+1 are emitted before the AV matmuls of unit i, so the PE
never sits behind ScalarE's exp).  Projection and output-projection matmul
groups are generators fed one-matmul-at-a-time as fillers inside the
ACT-paced attention stretches: "hard" projection groups may run one chunk
early and force-drain before the chunk that needs them; "soft" outproj groups
carry staggered due-tags so qc's outproj feeds qc+1..qc+3's attention, with
the last two qc2 groups reserved for the final drain so the PE stays warm
(HAM K=8/8) across the last normalize chain instead of running the tail
output projection at 1.2 GHz.  AV psum pairs are allocated lazily at first-AV
emission so the av pool rotation stays deadlock-free around the broadcast
matmuls of normalize_half.
"""

import numpy as np
import ml_dtypes

B, T, C, H = 4, 2048, 1024, 16
HS = C // H          # 64
NHL = 8              # local heads per core
KT = C // 128        # 8 contraction subtiles
NQC = T // 512       # 4 query chunks
NTB = T // 128       # 16 t-blocks
Bb16 = ml_dtypes.bfloat16

_CACHE = {}


def _build():
    import concourse.bass as bass
    import concourse.bacc as bacc
    import concourse.tile as tile
    import concourse.mybir as mybir
    from collections import deque

    BF = mybir.dt.bfloat16
    F32 = mybir.dt.float32
    AF = mybir.ActivationFunctionType

    nc = bacc.Bacc("TRN2", target_bir_lowering=False, debug=False, num_devices=8)
    xT = nc.dram_tensor("xT", [C, T], BF, kind="ExternalInput").ap()
    wqk = nc.dram_tensor("wqk", [C, 1024], BF, kind="ExternalInput").ap()
    wv = nc.dram_tensor("wv", [C, 520], BF, kind="ExternalInput").ap()
    wp = nc.dram_tensor("wp", [512, C], BF, kind="ExternalInput").ap()
    mask = nc.dram_tensor("mask", [128, 1280], BF, kind="ExternalInput").ap()
    out = nc.dram_tensor("out", [T, C], BF, kind="ExternalOutput").ap()

    MOFF = [0, 512, 896, 1152]   # mask_sb offsets for diag blocks d=0..3

    with tile.TileContext(nc) as tc:
        with tc.tile_pool(name="persist", bufs=1) as persist, \
             tc.tile_pool(name="mm", bufs=2, space="PSUM") as mmpool, \
             tc.tile_pool(name="s", bufs=2, space="PSUM") as spool, \
             tc.tile_pool(name="av", bufs=2, space="PSUM") as avpool, \
             tc.tile_pool(name="e", bufs=6) as epool, \
             tc.tile_pool(name="nrm", bufs=3) as nrmpool, \
             tc.tile_pool(name="osb", bufs=3) as outpool:

            xT_sb = persist.tile([128, KT, T], BF, tag="xT")
            wqk_sb = persist.tile([128, KT, 1024], BF, tag="wqk")
            wv_sb = persist.tile([128, KT, 520], BF, tag="wv")
            wp_sb = persist.tile([128, 4, 1024], BF, tag="wp")
            mask_sb = persist.tile([128, 1280], BF, tag="mask")
            qk_sb = persist.tile([128, 8, T], BF, tag="qk")
            v_sb = persist.tile([128, NTB, 520], BF, tag="v")
            yT_sb = persist.tile([128, 4, T], BF, tag="yT")
            # pair-broadcast weights: w[0,0:64]=w[64,0:64]=1 selects rA
            # into psum partitions 0:64, w[32,64:128]=w[96,64:128]=1 selects rB
            # into 64:128; K=33 slices at base 0 / base 64 match r4's rows
            ones33_sb = persist.tile([97, 128], F32, tag="ones33")
            nc.vector.memset(ones33_sb[:], 0.0)
            for r in (0, 64):
                nc.vector.memset(ones33_sb[r:r + 1, 0:64], 1.0)
                nc.vector.memset(ones33_sb[r + 32:r + 33, 64:128], 1.0)

            # load order: exactly what the first projection chunk needs, first
            for k in range(KT):
                nc.sync.dma_start(wqk_sb[:, k, :], wqk[k * 128:(k + 1) * 128, :])
                nc.sync.dma_start(xT_sb[:, k, 0:512], xT[k * 128:(k + 1) * 128, 0:512])
            for k in range(KT):
                nc.sync.dma_start(wv_sb[:, k, :], wv[k * 128:(k + 1) * 128, :])
            nc.sync.dma_start(mask_sb[:], mask[:])
            for k in range(KT):
                nc.sync.dma_start(xT_sb[:, k, 512:1024],
                                  xT[k * 128:(k + 1) * 128, 512:1024])
            for k in range(4):
                nc.sync.dma_start(wp_sb[:, k, :], wp[k * 128:(k + 1) * 128, :])
            for k in range(KT):
                nc.sync.dma_start(xT_sb[:, k, 1024:2048],
                                  xT[k * 128:(k + 1) * 128, 1024:2048])

            # ---- filler generators: ~one PE matmul per step ----
            def qk_group_gen(qc, m):
                q0 = qc * 512
                mm_ps = mmpool.tile([128, 512], F32, tag="mm",
                                    name=f"qkg_{qc}_{m}")
                for k in range(KT):
                    nc.tensor.matmul(
                        mm_ps[:], wqk_sb[:, k, m * 128:(m + 1) * 128],
                        xT_sb[:, k, q0:q0 + 512],
                        start=(k == 0), stop=(k == KT - 1))
                    yield
                nc.vector.tensor_copy(qk_sb[:, m, q0:q0 + 512], mm_ps[:])
                yield

            def v_group_gen(j):
                jj = j * 128
                vps = mmpool.tile([128, 512], F32, tag="mm", name=f"vg_{j}")
                for k in range(KT):
                    nc.tensor.matmul(
                        vps[:], xT_sb[:, k, jj:jj + 128],
                        wv_sb[:, k, 0:512],
                        start=(k == 0), stop=(k == KT - 1))
                    yield
                nc.vector.tensor_copy(v_sb[:, j, 0:512], vps[:])
                yield
                vps2 = mmpool.tile([128, 8], F32, tag="mm", name=f"vg2_{j}")
                for k in range(KT):
                    nc.tensor.matmul(
                        vps2[:], xT_sb[:, k, jj:jj + 128],
                        wv_sb[:, k, 512:520],
                        start=(k == 0), stop=(k == KT - 1))
                    if k % 4 == 3:
                        yield
                nc.vector.tensor_copy(v_sb[:, j, 512:520], vps2[:])
                vones = v_sb[:, j, :].rearrange("p (h e) -> p h e", e=65)[:, :, 64]
                nc.vector.memset(vones, 1.0)
                yield

            def outproj_group_gen(qc, tt):
                t0 = (qc * 4 + tt) * 128
                osb = outpool.tile([128, 1024], BF, tag="osb", name=f"og_{qc}_{tt}")
                for n in range(2):
                    ops = mmpool.tile([128, 512], F32, tag="mm",
                                      name=f"op_{qc}_{tt}_{n}")
                    for cp in range(4):
                        nc.tensor.matmul(
                            ops[:], yT_sb[:, cp, t0:t0 + 128],
                            wp_sb[:, cp, n * 512:(n + 1) * 512],
                            start=(cp == 0), stop=(cp == 3))
                        yield
                    with nc.allow_low_precision(reason="bf16 partial output"):
                        nc.vector.tensor_copy(osb[:, n * 512:(n + 1) * 512], ops[:])
                    yield
                nc.sync.dma_start(out[t0:t0 + 128, :], osb[:])

            def proj_gens(qc):
                return [(qc, qk_group_gen(qc, m)) for m in range(8)] + \
                       [(qc, v_group_gen(j)) for j in range(4 * qc, 4 * qc + 4)]

            fillers = deque()        # hard: must complete before their qc
            soft = deque()           # outproj: fed inside attention only

            def feed(now, n):
                # prefer soft work (outproj) inside attention; hard proj
                # groups otherwise drain in bulk at qc boundaries anyway
                while n > 0 and (soft or fillers):
                    if soft and soft[0][0] <= now:
                        q = soft
                    elif fillers and fillers[0][0] <= now + 1:
                        # hard proj groups may run one chunk early
                        q = fillers
                    else:
                        return
                    tag, g = q[0]
                    try:
                        next(g)
                        n -= 1
                    except StopIteration:
                        q.popleft()

            def drain(now):
                while fillers and fillers[0][0] <= now:
                    tag, g = fillers.popleft()
                    for _ in g:
                        pass

            def drain_soft():
                while soft:
                    tag, g = soft.popleft()
                    for _ in g:
                        pass

            # ---- attention units: one kv block, both heads of a pair ----
            def unit_list(qc):
                units = []
                for hp in range(4):
                    blocks = [(j, 0, 512) for j in range(4 * qc)] + \
                             [(4 * qc + d, 128 * d, 512 - 128 * d) for d in range(4)]
                    nb = len(blocks)
                    for bi, (j, qo, w) in enumerate(blocks):
                        units.append((hp, j, qo, w, bi == 0, bi == nb - 1))
                return units

            def emit_S(qc, u):
                hp, j, qo, w, first, last = u
                q0 = qc * 512
                s = spool.tile([128, 1024], F32, tag="s",
                               name=f"s_{qc}_{hp}_{j}")
                # the two heads target disjoint PE row groups, so they stream
                # through the array concurrently
                for pb, off in ((0, 0), (64, 512)):
                    nc.tensor.matmul(
                        s[:, off:off + w],
                        qk_sb[pb:pb + 64, 4 + hp, j * 128:(j + 1) * 128],
                        qk_sb[pb:pb + 64, hp, q0 + qo:q0 + 512],
                        start=True, stop=True,
                        tile_position=(pb, 0))
                return s

            def emit_expmask(qc, u, s):
                hp, j, qo, w, first, last = u
                e = epool.tile([128, 1024], BF, tag="e", name=f"e_{qc}_{hp}_{j}")
                s3 = s.rearrange("p (g q) -> p g q", g=2)[:, :, 0:w]
                e3 = e.rearrange("p (g q) -> p g q", g=2)[:, :, 0:w]
                nc.scalar.activation(e3, s3, AF.Exp, scale=0.125)
                if j >= 4 * qc:
                    moff = MOFF[j - 4 * qc]
                    for off in (0, 512):
                        nc.vector.tensor_mul(
                            e[:, off:off + w], e[:, off:off + w],
                            mask_sb[:, moff:moff + w])
                return e

            def emit_AV(qc, u, e, avA, avB):
                hp, j, qo, w, first, last = u
                for av, off, h in ((avA, 0, 2 * hp), (avB, 512, 2 * hp + 1)):
                    nc.tensor.matmul(
                        av[:, qo:512], v_sb[:, j, h * 65:h * 65 + 65],
                        e[:, off:off + w],
                        start=first, stop=last)

            def pair_end(qc, hp, avA, avB, yraw_sb, den8_sb):
                # stash y and denominator; av psum slots free right away
                for h, av_ps in ((2 * hp, avA), (2 * hp + 1, avB)):
                    with nc.allow_low_precision(reason="attention y bf16"):
                        nc.vector.tensor_copy(yraw_sb[:, h, :], av_ps[0:64, :])
                    p32 = (h % 4) * 32
                    nc.vector.tensor_copy(
                        den8_sb[p32:p32 + 1, h // 4, :], av_ps[64:65, :])

            def normalize_half(qc, half, yraw_sb, den8_sb):
                # heads 4*half .. 4*half+3 finished: reciprocal + scale them.
                # One K=33 fp32 matmul per head-pair broadcasts both heads'
                # reciprocal rows straight out of r4 (rows 1-31 are finite 1.0
                # from the den8 memset, zero-weighted).
                q0 = qc * 512
                r4_sb = nrmpool.tile([128, 512], F32, tag="r4",
                                     name=f"r4_{qc}_{half}")
                nc.vector.reciprocal_approx_fast(r4_sb[:], den8_sb[:, half, :])
                for hp in (2 * half, 2 * half + 1):
                    base = (hp % 2) * 64
                    bc_ps = avpool.tile([128, 512], F32, tag="av",
                                        name=f"bc_{qc}_{hp}")
                    with nc.allow_low_precision(reason="fp32r broadcast"):
                        nc.tensor.matmul(
                            bc_ps[:], ones33_sb[base:base + 33, :],
                            r4_sb[base:base + 33, :], start=True, stop=True)
                    with nc.allow_low_precision(reason="attention y bf16"):
                        nc.vector.tensor_mul(
                            yT_sb[0:64, hp, q0:q0 + 512],
                            yraw_sb[:, 2 * hp, :], bc_ps[0:64, :])
                        nc.vector.tensor_mul(
                            yT_sb[64:128, hp, q0:q0 + 512],
                            yraw_sb[:, 2 * hp + 1, :], bc_ps[64:128, :])

            # ---- main schedule ----
            # boot: chunk-0 qT/kT projection with k as the OUTER loop for the
            # first 6 column slots (2 mm bufs + 4 s-tile halves live at once,
            # so the first matmuls issue after only the first k-slice of DMA);
            # slots 6,7 run k-inner through the recycled mm buffers.
            ps = []
            for m in range(6):
                if m < 2:
                    ps.append(mmpool.tile([128, 512], F32, tag="mm",
                                          name=f"boot{m}"))
                else:
                    if m % 2 == 0:
                        st = spool.tile([128, 1024], F32, tag="s",
                                        name=f"boot{m}")
                    ps.append(st[:, (m % 2) * 512:(m % 2) * 512 + 512])
            for k in range(KT):
                for m in range(6):
                    nc.tensor.matmul(
                        ps[m], wqk_sb[:, k, m * 128:(m + 1) * 128],
                        xT_sb[:, k, 0:512],
                        start=(k == 0), stop=(k == KT - 1))
            for m in range(6):
                nc.vector.tensor_copy(qk_sb[:, m, 0:512], ps[m])
            # k-slots for head-pairs 2,3 are not needed until halfway
            # through qc0's attention: feed them as fillers instead of
            # running them before the first S
            for _, g in proj_gens(0)[8:]:
                for _ in g:
                    pass
            fillers.extend((1, qk_group_gen(0, m)) for m in (6, 7))
            fillers.extend(proj_gens(1))

            for qc in range(NQC):
                drain(qc)
                yraw_sb = nrmpool.tile([64, NHL, 512], BF, tag="yraw",
                                       name=f"yraw{qc}")
                den8_sb = nrmpool.tile([128, 2, 512], F32, tag="den8",
                                       name=f"den8{qc}")
                nc.vector.memset(den8_sb[:], 1.0)
                units = unit_list(qc)
                pend = None      # (unit, e)
                cur = {}

                def flush(pend):
                    u, e = pend
                    hp, j, qo, w, first, last = u
                    if first:
                        cur["avA"] = avpool.tile([65, 512], F32, tag="av",
                                                 name=f"avA_{qc}_{hp}")
                        cur["avB"] = avpool.tile([65, 512], F32, tag="av",
                                                 name=f"avB_{qc}_{hp}")
                    emit_AV(qc, u, e, cur["avA"], cur["avB"])
                    if last:
                        pair_end(qc, hp, cur["avA"], cur["avB"],
                                 yraw_sb, den8_sb)
                        if hp == 1:
                            normalize_half(qc, 0, yraw_sb, den8_sb)
                        elif hp == 3:
                            normalize_half(qc, 1, yraw_sb, den8_sb)

                for u in units:
                    s = emit_S(qc, u)
                    e = emit_expmask(qc, u, s)
                    if pend is not None:
                        flush(pend)
                    feed(qc, 2 if qc == 0 else 1)
                    pend = (u, e)
                flush(pend)
                # stagger outproj due-tags: most groups feed the next chunks'
                # attention; the last two of qc2 stay for the drain tail so
                # the PE stays warm across the final normalize chain
                otags = {0: (1, 1, 2, 3), 1: (2, 2, 3, 3),
                         2: (3, 3, 3, 4), 3: (4, 4, 4, 4)}[qc]
                soft.extend((otags[tt], outproj_group_gen(qc, tt))
                            for tt in range(4))
                if qc + 2 < NQC:
                    fillers.extend(proj_gens(qc + 2))
            drain(NQC)
            drain_soft()
    nc.compile()
    return nc


def _get_nc():
    if "nc" not in _CACHE:
        _CACHE["nc"] = _build()
    return _CACHE["nc"]


def _host_prep(x, W_attn, W_proj):
    """Shard + lay out per-core inputs. Returns list of 8 in_maps."""
    x = np.asarray(x, dtype=np.float32)
    W_attn = np.asarray(W_attn, dtype=np.float32)
    W_proj = np.asarray(W_proj, dtype=np.float32)

    # triangular mask prefix: mask[s, i] = 1.0 if s <= i else 0
    s_idx = np.arange(128)[:, None]
    q_idx = np.arange(512)[None, :]
    tri = (s_idx <= q_idx).astype(np.float32)
    mask = np.ascontiguousarray(np.concatenate(
        [tri[:, :512], tri[:, :384], tri[:, :256], tri[:, :128]], axis=1
    )).astype(Bb16)

    xT_b = [np.ascontiguousarray(x[b].T).astype(Bb16) for b in range(B)]
    in_maps = []
    for core in range(8):
        b, g = core // 2, core % 2
        c0 = g * 512
        wqk_g = np.concatenate(
            [W_attn[:, c0:c0 + 512], W_attn[:, C + c0:C + c0 + 512]], axis=1
        ).astype(Bb16)
        vbase = W_attn[:, 2 * C + c0:2 * C + c0 + 512]
        wv_g = np.zeros((C, 520), dtype=np.float32)
        for h in range(NHL):
            wv_g[:, h * 65:h * 65 + 64] = vbase[:, h * 64:(h + 1) * 64]
        wp_g = np.ascontiguousarray(W_proj[c0:c0 + 512, :]).astype(Bb16)
        in_maps.append({
            "xT": xT_b[b],
            "wqk": np.ascontiguousarray(wqk_g),
            "wv": wv_g.astype(Bb16),
            "wp": wp_g,
            "mask": mask,
        })
    return in_maps


def kernel(x, W_attn, W_proj):
    from concourse import bass_utils

    nc = _get_nc()
    in_maps = _host_prep(x, W_attn, W_proj)
    res = bass_utils.run_bass_kernel_spmd(nc, in_maps, core_ids=list(range(8)))
    outs = [res.results[c]["out"] for c in range(8)]
    full = np.empty((B, T, C), dtype=np.float32)
    for b in range(B):
        full[b] = outs[2 * b].astype(np.float32) + outs[2 * b + 1].astype(np.float32)
    return full


# revision 30
# speedup vs baseline: 1.0025x; 1.0025x over previous
"""Causal multi-head self-attention (B=4, T=2048, C=1024, H=16) on 8 TRN2 NeuronCores.

Sharding: core = b*2 + g  (b = batch 0..3, g = head-group 0..1 of 8 heads each).
Data parallel over batch; tensor parallel over heads (column-parallel W_attn,
row-parallel W_proj). Each core returns a partial (T, C) output; the host sums
the two partials per batch (the TP all-reduce happens in the unshard step).

Per-core device kernel (bf16 matmuls, f32 accumulation), per 512-wide q chunk:
  1. qT/kT projection with heads on partitions; head pairs share a 128-row tile
  2. v_aug projection in natural [t, c] layout with an all-ones column per head
     (the ones column turns the softmax denominator into row 64 of the y^T psum)
  3. attention in transposed [s, q] layout -- no transposes anywhere:
       S^T block = (kT block)^T @ qT chunk  (head-even rows 0:64 / head-odd rows
       64:128 of the PE array run concurrently: disjoint row groups)
       E = exp(S/8) on ScalarE (no max-subtraction: |scores|/8 < ~7)
       causal mask = precomputed multiplicative 0/1 tile on diagonal blocks
       y^T_aug accumulates v_aug^T @ E over s blocks in PSUM
  4. row-parallel output projection of the finished q chunk.

v2 scheduling: the attention loop runs in per-kv-block units holding one
[128, 1024] S psum tile (head A in cols 0:512, head B in 512:1024, one exp
ACT instruction covers both).  Units are software-pipelined one ahead (the
S matmuls of unit i+1 are emitted before the AV matmuls of unit i, so the PE
never sits behind ScalarE's exp).  Projection and output-projection matmul
groups are generators fed one-matmul-at-a-time as fillers inside the
ACT-paced attention stretches: "hard" projection groups may run one chunk
early and force-drain before the chunk that needs them; "soft" outproj groups
carry staggered due-tags so qc's outproj feeds qc+1..qc+3's attention, with
the last two qc2 groups reserved for the final drain so the PE stays warm
(HAM K=8/8) across the last normalize chain instead of running the tail
output projection at 1.2 GHz.  AV psum pairs are allocated lazily at first-AV
emission so the av pool rotation stays deadlock-free around the broadcast
matmuls of normalize_half.
"""

import numpy as np
import ml_dtypes

B, T, C, H = 4, 2048, 1024, 16
HS = C // H          # 64
NHL = 8              # local heads per core
KT = C // 128        # 8 contraction subtiles
NQC = T // 512       # 4 query chunks
NTB = T // 128       # 16 t-blocks
Bb16 = ml_dtypes.bfloat16

_CACHE = {}


def _build():
    import concourse.bass as bass
    import concourse.bacc as bacc
    import concourse.tile as tile
    import concourse.mybir as mybir
    from collections import deque

    BF = mybir.dt.bfloat16
    F32 = mybir.dt.float32
    AF = mybir.ActivationFunctionType

    nc = bacc.Bacc("TRN2", target_bir_lowering=False, debug=False, num_devices=8)
    xT = nc.dram_tensor("xT", [C, T], BF, kind="ExternalInput").ap()
    wqk = nc.dram_tensor("wqk", [C, 1024], BF, kind="ExternalInput").ap()
    wv = nc.dram_tensor("wv", [C, 520], BF, kind="ExternalInput").ap()
    wp = nc.dram_tensor("wp", [512, C], BF, kind="ExternalInput").ap()
    mask = nc.dram_tensor("mask", [128, 1280], BF, kind="ExternalInput").ap()
    out = nc.dram_tensor("out", [T, C], BF, kind="ExternalOutput").ap()

    MOFF = [0, 512, 896, 1152]   # mask_sb offsets for diag blocks d=0..3

    with tile.TileContext(nc) as tc:
        with tc.tile_pool(name="persist", bufs=1) as persist, \
             tc.tile_pool(name="mm", bufs=2, space="PSUM") as mmpool, \
             tc.tile_pool(name="s", bufs=2, space="PSUM") as spool, \
             tc.tile_pool(name="av", bufs=2, space="PSUM") as avpool, \
             tc.tile_pool(name="e", bufs=6) as epool, \
             tc.tile_pool(name="nrm", bufs=3) as nrmpool, \
             tc.tile_pool(name="osb", bufs=3) as outpool:

            xT_sb = persist.tile([128, KT, T], BF, tag="xT")
            wqk_sb = persist.tile([128, KT, 1024], BF, tag="wqk")
            wv_sb = persist.tile([128, KT, 520], BF, tag="wv")
            wp_sb = persist.tile([128, 4, 1024], BF, tag="wp")
            mask_sb = persist.tile([128, 1280], BF, tag="mask")
            qk_sb = persist.tile([128, 8, T], BF, tag="qk")
            v_sb = persist.tile([128, NTB, 520], BF, tag="v")
            yT_sb = persist.tile([128, 4, T], BF, tag="yT")
            # pair-broadcast weights: w[0,0:64]=w[64,0:64]=1 selects rA
            # into psum partitions 0:64, w[32,64:128]=w[96,64:128]=1 selects rB
            # into 64:128; K=33 slices at base 0 / base 64 match r4's rows
            ones33_sb = persist.tile([97, 128], F32, tag="ones33")
            nc.vector.memset(ones33_sb[:], 0.0)
            for r in (0, 64):
                nc.vector.memset(ones33_sb[r:r + 1, 0:64], 1.0)
                nc.vector.memset(ones33_sb[r + 32:r + 33, 64:128], 1.0)

            # load order: exactly what the first projection chunk needs, first
            for k in range(KT):
                nc.sync.dma_start(wqk_sb[:, k, :], wqk[k * 128:(k + 1) * 128, :])
                nc.sync.dma_start(xT_sb[:, k, 0:512], xT[k * 128:(k + 1) * 128, 0:512])
            for k in range(KT):
                nc.sync.dma_start(wv_sb[:, k, :], wv[k * 128:(k + 1) * 128, :])
            nc.sync.dma_start(mask_sb[:], mask[:])
            for k in range(KT):
                nc.sync.dma_start(xT_sb[:, k, 512:1024],
                                  xT[k * 128:(k + 1) * 128, 512:1024])
            for k in range(4):
                nc.sync.dma_start(wp_sb[:, k, :], wp[k * 128:(k + 1) * 128, :])
            for k in range(KT):
                nc.sync.dma_start(xT_sb[:, k, 1024:2048],
                                  xT[k * 128:(k + 1) * 128, 1024:2048])

            # ---- filler generators: ~one PE matmul per step ----
            def qk_group_gen(qc, m):
                q0 = qc * 512
                mm_ps = mmpool.tile([128, 512], F32, tag="mm",
                                    name=f"qkg_{qc}_{m}")
                for k in range(KT):
                    nc.tensor.matmul(
                        mm_ps[:], wqk_sb[:, k, m * 128:(m + 1) * 128],
                        xT_sb[:, k, q0:q0 + 512],
                        start=(k == 0), stop=(k == KT - 1))
                    yield
                nc.vector.tensor_copy(qk_sb[:, m, q0:q0 + 512], mm_ps[:])
                yield

            def v_group_gen(j):
                jj = j * 128
                vps = mmpool.tile([128, 512], F32, tag="mm", name=f"vg_{j}")
                for k in range(KT):
                    nc.tensor.matmul(
                        vps[:], xT_sb[:, k, jj:jj + 128],
                        wv_sb[:, k, 0:512],
                        start=(k == 0), stop=(k == KT - 1))
                    yield
                nc.vector.tensor_copy(v_sb[:, j, 0:512], vps[:])
                yield
                vps2 = mmpool.tile([128, 8], F32, tag="mm", name=f"vg2_{j}")
                for k in range(KT):
                    nc.tensor.matmul(
                        vps2[:], xT_sb[:, k, jj:jj + 128],
                        wv_sb[:, k, 512:520],
                        start=(k == 0), stop=(k == KT - 1))
                    if k % 4 == 3:
                        yield
                nc.vector.tensor_copy(v_sb[:, j, 512:520], vps2[:])
                vones = v_sb[:, j, :].rearrange("p (h e) -> p h e", e=65)[:, :, 64]
                nc.vector.memset(vones, 1.0)
                yield

            def outproj_group_gen(qc, tt):
                t0 = (qc * 4 + tt) * 128
                osb = outpool.tile([128, 1024], BF, tag="osb", name=f"og_{qc}_{tt}")
                for n in range(2):
                    ops = mmpool.tile([128, 512], F32, tag="mm",
                                      name=f"op_{qc}_{tt}_{n}")
                    for cp in range(4):
                        nc.tensor.matmul(
                            ops[:], yT_sb[:, cp, t0:t0 + 128],
                            wp_sb[:, cp, n * 512:(n + 1) * 512],
                            start=(cp == 0), stop=(cp == 3))
                        yield
                    with nc.allow_low_precision(reason="bf16 partial output"):
                        nc.vector.tensor_copy(osb[:, n * 512:(n + 1) * 512], ops[:])
                    yield
                nc.sync.dma_start(out[t0:t0 + 128, :], osb[:])

            def proj_gens(qc):
                return [(qc, qk_group_gen(qc, m)) for m in range(8)] + \
                       [(qc, v_group_gen(j)) for j in range(4 * qc, 4 * qc + 4)]

            fillers = deque()        # hard: must complete before their qc
            soft = deque()           # outproj: fed inside attention only

            def feed(now, n):
                # prefer soft work (outproj) inside attention; hard proj
                # groups otherwise drain in bulk at qc boundaries anyway
                while n > 0 and (soft or fillers):
                    if soft and soft[0][0] <= now:
                        q = soft
                    elif fillers and fillers[0][0] <= now + 1:
                        # hard proj groups may run one chunk early
                        q = fillers
                    else:
                        return
                    tag, g = q[0]
                    try:
                        next(g)
                        n -= 1
                    except StopIteration:
                        q.popleft()

            def drain(now):
                while fillers and fillers[0][0] <= now:
                    tag, g = fillers.popleft()
                    for _ in g:
                        pass

            def drain_soft():
                while soft:
                    tag, g = soft.popleft()
                    for _ in g:
                        pass

            # ---- attention units: one kv block, both heads of a pair ----
            def unit_list(qc):
                units = []
                for hp in range(4):
                    blocks = [(j, 0, 512) for j in range(4 * qc)] + \
                             [(4 * qc + d, 128 * d, 512 - 128 * d) for d in range(4)]
                    nb = len(blocks)
                    for bi, (j, qo, w) in enumerate(blocks):
                        units.append((hp, j, qo, w, bi == 0, bi == nb - 1))
                return units

            def emit_S(qc, u):
                hp, j, qo, w, first, last = u
                q0 = qc * 512
                s = spool.tile([128, 1024], F32, tag="s",
                               name=f"s_{qc}_{hp}_{j}")
                # the two heads target disjoint PE row groups, so they stream
                # through the array concurrently
                for pb, off in ((0, 0), (64, 512)):
                    nc.tensor.matmul(
                        s[:, off:off + w],
                        qk_sb[pb:pb + 64, 4 + hp, j * 128:(j + 1) * 128],
                        qk_sb[pb:pb + 64, hp, q0 + qo:q0 + 512],
                        start=True, stop=True,
                        tile_position=(pb, 0))
                return s

            def emit_expmask(qc, u, s):
                hp, j, qo, w, first, last = u
                e = epool.tile([128, 1024], BF, tag="e", name=f"e_{qc}_{hp}_{j}")
                s3 = s.rearrange("p (g q) -> p g q", g=2)[:, :, 0:w]
                e3 = e.rearrange("p (g q) -> p g q", g=2)[:, :, 0:w]
                nc.scalar.activation(e3, s3, AF.Exp, scale=0.125)
                if j >= 4 * qc:
                    moff = MOFF[j - 4 * qc]
                    for off in (0, 512):
                        nc.vector.tensor_mul(
                            e[:, off:off + w], e[:, off:off + w],
                            mask_sb[:, moff:moff + w])
                return e

            def emit_AV(qc, u, e, avA, avB):
                hp, j, qo, w, first, last = u
                for av, off, h in ((avA, 0, 2 * hp), (avB, 512, 2 * hp + 1)):
                    nc.tensor.matmul(
                        av[:, qo:512], v_sb[:, j, h * 65:h * 65 + 65],
                        e[:, off:off + w],
                        start=first, stop=last)

            def pair_end(qc, hp, avA, avB, yraw_sb, den8_sb):
                # stash y and denominator; av psum slots free right away
                for h, av_ps in ((2 * hp, avA), (2 * hp + 1, avB)):
                    with nc.allow_low_precision(reason="attention y bf16"):
                        nc.vector.tensor_copy(yraw_sb[:, h, :], av_ps[0:64, :])
                    p32 = (h % 4) * 32
                    nc.vector.tensor_copy(
                        den8_sb[p32:p32 + 1, h // 4, :], av_ps[64:65, :])

            def normalize_half(qc, half, yraw_sb, den8_sb):
                # heads 4*half .. 4*half+3 finished: reciprocal + scale them.
                # One K=33 fp32 matmul per head-pair broadcasts both heads'
                # reciprocal rows straight out of r4 (rows 1-31 are finite 1.0
                # from the den8 memset, zero-weighted).
                q0 = qc * 512
                r4_sb = nrmpool.tile([128, 512], F32, tag="r4",
                                     name=f"r4_{qc}_{half}")
                nc.vector.reciprocal_approx_fast(r4_sb[:], den8_sb[:, half, :])
                for hp in (2 * half, 2 * half + 1):
                    base = (hp % 2) * 64
                    bc_ps = avpool.tile([128, 512], F32, tag="av",
                                        name=f"bc_{qc}_{hp}")
                    with nc.allow_low_precision(reason="fp32r broadcast"):
                        nc.tensor.matmul(
                            bc_ps[:], ones33_sb[base:base + 33, :],
                            r4_sb[base:base + 33, :], start=True, stop=True)
                    with nc.allow_low_precision(reason="attention y bf16"):
                        nc.vector.tensor_mul(
                            yT_sb[0:64, hp, q0:q0 + 512],
                            yraw_sb[:, 2 * hp, :], bc_ps[0:64, :])
                        nc.vector.tensor_mul(
                            yT_sb[64:128, hp, q0:q0 + 512],
                            yraw_sb[:, 2 * hp + 1, :], bc_ps[64:128, :])

            # ---- main schedule ----
            # boot: chunk-0 qT/kT projection with k as the OUTER loop for the
            # first 6 column slots (2 mm bufs + 4 s-tile halves live at once,
            # so the first matmuls issue after only the first k-slice of DMA);
            # slots 6,7 run k-inner through the recycled mm buffers.
            ps = []
            for m in range(6):
                if m < 2:
                    ps.append(mmpool.tile([128, 512], F32, tag="mm",
                                          name=f"boot{m}"))
                else:
                    if m % 2 == 0:
                        st = spool.tile([128, 1024], F32, tag="s",
                                        name=f"boot{m}")
                    ps.append(st[:, (m % 2) * 512:(m % 2) * 512 + 512])
            for k in range(KT):
                for m in range(6):
                    nc.tensor.matmul(
                        ps[m], wqk_sb[:, k, m * 128:(m + 1) * 128],
                        xT_sb[:, k, 0:512],
                        start=(k == 0), stop=(k == KT - 1))
            for m in range(6):
                nc.vector.tensor_copy(qk_sb[:, m, 0:512], ps[m])
            # k-slots for head-pairs 2,3 are not needed until halfway
            # through qc0's attention: feed them as fillers instead of
            # running them before the first S
            for _, g in proj_gens(0)[8:]:
                for _ in g:
                    pass
            fillers.extend((1, qk_group_gen(0, m)) for m in (6, 7))
            fillers.extend(proj_gens(1))

            for qc in range(NQC):
                drain(qc)
                yraw_sb = nrmpool.tile([64, NHL, 512], BF, tag="yraw",
                                       name=f"yraw{qc}")
                den8_sb = nrmpool.tile([128, 2, 512], F32, tag="den8",
                                       name=f"den8{qc}")
                nc.vector.memset(den8_sb[:], 1.0)
                units = unit_list(qc)
                pend = None      # (unit, e)
                cur = {}

                def flush(pend):
                    u, e = pend
                    hp, j, qo, w, first, last = u
                    if first:
                        cur["avA"] = avpool.tile([65, 512], F32, tag="av",
                                                 name=f"avA_{qc}_{hp}")
                        cur["avB"] = avpool.tile([65, 512], F32, tag="av",
                                                 name=f"avB_{qc}_{hp}")
                    emit_AV(qc, u, e, cur["avA"], cur["avB"])
                    if last:
                        pair_end(qc, hp, cur["avA"], cur["avB"],
                                 yraw_sb, den8_sb)
                        if hp == 1:
                            normalize_half(qc, 0, yraw_sb, den8_sb)
                        elif hp == 3:
                            normalize_half(qc, 1, yraw_sb, den8_sb)

                # units in twos: both S pairs back-to-back so the second
                # unit's LDWEIGHTS overlaps the first's member-B stream
                # (disjoint row groups); pairs have an even unit count so
                # groups never straddle an hp boundary
                for ua, ub in zip(units[0::2], units[1::2]):
                    sa = emit_S(qc, ua)
                    sb = emit_S(qc, ub)
                    ea = emit_expmask(qc, ua, sa)
                    eb = emit_expmask(qc, ub, sb)
                    if pend is not None:
                        flush(pend[0])
                        flush(pend[1])
                    feed(qc, 4 if qc == 0 else 2)
                    pend = ((ua, ea), (ub, eb))
                flush(pend[0])
                flush(pend[1])
                # stagger outproj due-tags: most groups feed the next chunks'
                # attention; the last two of qc2 stay for the drain tail so
                # the PE stays warm across the final normalize chain
                otags = {0: (1, 1, 2, 3), 1: (2, 2, 3, 3),
                         2: (3, 3, 3, 4), 3: (4, 4, 4, 4)}[qc]
                soft.extend((otags[tt], outproj_group_gen(qc, tt))
                            for tt in range(4))
                if qc + 2 < NQC:
                    fillers.extend(proj_gens(qc + 2))
            drain(NQC)
            drain_soft()
    nc.compile()
    return nc


def _get_nc():
    if "nc" not in _CACHE:
        _CACHE["nc"] = _build()
    return _CACHE["nc"]


def _host_prep(x, W_attn, W_proj):
    """Shard + lay out per-core inputs. Returns list of 8 in_maps."""
    x = np.asarray(x, dtype=np.float32)
    W_attn = np.asarray(W_attn, dtype=np.float32)
    W_proj = np.asarray(W_proj, dtype=np.float32)

    # triangular mask prefix: mask[s, i] = 1.0 if s <= i else 0
    s_idx = np.arange(128)[:, None]
    q_idx = np.arange(512)[None, :]
    tri = (s_idx <= q_idx).astype(np.float32)
    mask = np.ascontiguousarray(np.concatenate(
        [tri[:, :512], tri[:, :384], tri[:, :256], tri[:, :128]], axis=1
    )).astype(Bb16)

    xT_b = [np.ascontiguousarray(x[b].T).astype(Bb16) for b in range(B)]
    in_maps = []
    for core in range(8):
        b, g = core // 2, core % 2
        c0 = g * 512
        wqk_g = np.concatenate(
            [W_attn[:, c0:c0 + 512], W_attn[:, C + c0:C + c0 + 512]], axis=1
        ).astype(Bb16)
        vbase = W_attn[:, 2 * C + c0:2 * C + c0 + 512]
        wv_g = np.zeros((C, 520), dtype=np.float32)
        for h in range(NHL):
            wv_g[:, h * 65:h * 65 + 64] = vbase[:, h * 64:(h + 1) * 64]
        wp_g = np.ascontiguousarray(W_proj[c0:c0 + 512, :]).astype(Bb16)
        in_maps.append({
            "xT": xT_b[b],
            "wqk": np.ascontiguousarray(wqk_g),
            "wv": wv_g.astype(Bb16),
            "wp": wp_g,
            "mask": mask,
        })
    return in_maps


def kernel(x, W_attn, W_proj):
    from concourse import bass_utils

    nc = _get_nc()
    in_maps = _host_prep(x, W_attn, W_proj)
    res = bass_utils.run_bass_kernel_spmd(nc, in_maps, core_ids=list(range(8)))
    outs = [res.results[c]["out"] for c in range(8)]
    full = np.empty((B, T, C), dtype=np.float32)
    for b in range(B):
        full[b] = outs[2 * b].astype(np.float32) + outs[2 * b + 1].astype(np.float32)
    return full
